# revision 32
# baseline (speedup 1.0000x reference)
"""Multi-head attention with RoPE on 8 Trainium2 NeuronCores.

Strategy: tensor-parallel over heads (16 heads / 8 cores = 2 heads per
core). Each core computes its 2 heads' q/k/v projections, RoPE, full
(non-causal) softmax attention, and a partial output projection over its
128-dim slice of the attention output; the host sums the 8 partial
outputs and adds the output bias.

Schedule/precision notes:
 - Keys are processed in groups of 256, parity-split (tokens are
   host-permuted so each 256-token group is stored evens-then-odds).
   The 4 score matmuls of a group land in one 4-bank PSUM tile and are
   consumed by a single 2048-element exp on the ACT engine (fewer,
   larger activations).
 - PV lags the exp stream by 2 groups; projections for the next batch
   and the out-projection of the previous chunk are sliced into the
   window of each group so the PE never idles on ACT.
 - scores are computed transposed (S^T [tk, tq]) so exp(S^T) feeds the
   PV matmul directly.  Softmax max-subtraction is skipped (scores are
   O(1)); the denominator comes from a ones column appended to V, is
   broadcast via a small selector matmul, and the reciprocal multiply
   commutes with the out-projection.
 - q/k projections run in fp8e4m3 with perf_mode=DoubleRow (256-deep
   contraction per matmul, halving those matmuls); the fp8 quantization
   noise only perturbs attention scores, where near-uniform softmax over
   2048 keys washes it out.  v / scores / PV / out-proj stay bf16 with
   fp32 PSUM accumulation.
 - q is stored zero-padded per head so score matmuls contract over the
   full 128 partitions in uniform 128x128 PE mode (row-tiled K=64 pairs
   measured slower: tiled LDWEIGHTS cannot overlap in-flight matmuls on
   the same row group).
 - output is written bf16 (halves write DMA); the host gather upcasts.
"""

import sys

for _p in ("/opt/trn_rl_repo",):
    if _p not in sys.path:
        sys.path.append(_p)

import numpy as np

import concourse.bacc as bacc
import concourse.bass as bass
import concourse.mybir as mybir
import concourse.tile as tile
from concourse.bass import _add_dep_helper
from concourse.bass_utils import run_bass_kernel_spmd

F32 = mybir.dt.float32
BF16 = mybir.dt.bfloat16
FP8 = mybir.dt.float8e4
MMDT = BF16
ALU = mybir.AluOpType

HIDDEN = 1024
HEADS = 16
D = 64
ROPE_BASE = 10000.0
NCORES = 8
HPC = HEADS // NCORES  # heads per core = 2
EPC = HPC * D  # out dims per core = 128
B_FULL, S_FULL = 4, 2048

TQ = 512  # query-chunk width
GK = 512  # key-group width (4 parities x 128)
NPAR = GK // 128  # parity chunks per key group
VW = 80  # v storage width per (group, parity, head) slot


import ml_dtypes


def to_mm(a):
    """Convert an fp32 array to the matmul operand dtype (bf16, RNE)."""
    return np.ascontiguousarray(np.asarray(a, np.float32).astype(ml_dtypes.bfloat16))


def to_fp8(a):
    """Convert an fp32 array to fp8e4m3 for DoubleRow matmuls."""
    return np.ascontiguousarray(
        np.asarray(a, np.float32).astype(ml_dtypes.float8_e4m3)
    )


# fp8 weight pre-scale: q/k projection weights are tiny (~1/32 / ~1/256
# after the attention scale), far below fp8e4m3's normal range.  Weights
# and biases are scaled up by S8 on the host; the rope cos/sin tables are
# divided by S8 so the rope output lands back at the true scale.
S8 = 256.0


def token_perm(T):
    """Per-256 group even/odd interleave permutation. perm[p] = source token."""
    return np.arange(T).reshape(-1, 128, 2).transpose(0, 2, 1).reshape(-1)


def build_nc(B, S):
    """Build the per-core Bass program (SPMD: all cores run this)."""
    nc = bacc.Bacc(None, target_bir_lowering=False)
    T = B * S
    NTQ = S // TQ  # query chunks per batch = 4
    NG = S // GK  # key groups per batch = 4
    NGC = TQ // GK  # key groups per query chunk = 1
    DC = HIDDEN // 128  # contraction chunks = 8

    xt_d = nc.dram_tensor("xt", [HIDDEN, T], MMDT, kind="ExternalInput")
    xt8_d = nc.dram_tensor("xt8", [HIDDEN, T], FP8, kind="ExternalInput")
    wq_d = nc.dram_tensor("wq", [HIDDEN, EPC], FP8, kind="ExternalInput")
    wk_d = nc.dram_tensor("wk", [HIDDEN, EPC], FP8, kind="ExternalInput")
    psw_d = nc.dram_tensor("psw", [EPC, EPC], MMDT, kind="ExternalInput")
    wv_d = nc.dram_tensor("wv", [HIDDEN, EPC], MMDT, kind="ExternalInput")
    wo_d = nc.dram_tensor("wo", [EPC, HIDDEN], MMDT, kind="ExternalInput")
    bq4_d = nc.dram_tensor("bq4", [EPC, 4], F32, kind="ExternalInput")
    bvb_d = nc.dram_tensor("bvb", [EPC, EPC], F32, kind="ExternalInput")
    c1_d = nc.dram_tensor("c1", [EPC, S], BF16, kind="ExternalInput")
    c2_d = nc.dram_tensor("c2", [EPC, S], BF16, kind="ExternalInput")
    sel2_d = nc.dram_tensor("sel2", [128, EPC], MMDT, kind="ExternalInput")
    yt_d = nc.dram_tensor("yt", [HIDDEN, T], BF16, kind="ExternalOutput")

    with tile.TileContext(nc) as tc:
        with (
            tc.tile_pool(name="const", bufs=1) as cpool,
            tc.tile_pool(name="xin", bufs=3) as xpool,
            tc.tile_pool(name="x8in", bufs=3) as x8pool,
            tc.tile_pool(name="qk", bufs=2) as qkpool,
            tc.tile_pool(name="vsb", bufs=2) as vpool,
            tc.tile_pool(name="esb", bufs=6) as epool,
            tc.tile_pool(name="work", bufs=2) as wpool,
            tc.tile_pool(name="yout", bufs=6) as ypool,
            tc.tile_pool(name="psA", bufs=2, space="PSUM") as psA,
            tc.tile_pool(name="psO", bufs=1, space="PSUM") as psO,
            tc.tile_pool(name="ps2", bufs=2, space="PSUM") as ps2,
        ):
            # ---- constants (resident all kernel) ----
            # wo/sel2 are loaded after the prologue emission (they are not
            # needed until the first out-projection ~25us in) so their DMA
            # does not compete with the startup-critical loads.
            def _ldw(dram, dt=MMDT):
                t = cpool.tile([128, DC, EPC], dt, tag=dram.name + "_sb")
                nc.sync.dma_start(t[:], dram[:, :].rearrange("(dc p) m -> p dc m", p=128))
                return t

            wq = _ldw(wq_d, FP8)
            bq4 = cpool.tile([EPC, 4], F32, tag="bq4_sb")
            nc.sync.dma_start(bq4[:], bq4_d[:, :])
            c1 = cpool.tile([EPC, S], BF16, tag="c1_sb")
            c2 = cpool.tile([EPC, S], BF16, tag="c2_sb")
            for s_ in range(4):
                cs_ = slice(s_ * S // 4, (s_ + 1) * S // 4)
                nc.sync.dma_start(c1[:, cs_], c1_d[:, cs_])
                nc.sync.dma_start(c2[:, cs_], c2_d[:, cs_])
            wk = _ldw(wk_d, FP8)
            wv = _ldw(wv_d)
            psw = cpool.tile([EPC, EPC], MMDT, tag="psw_sb")
            nc.sync.dma_start(psw[:], psw_d[:, :])
            bvb = cpool.tile([EPC, EPC], F32, tag="bvb_sb")
            nc.sync.dma_start(bvb[:], bvb_d[:, :])
            wo = [None]
            sel2 = [None]

            def load_tail_consts():
                wo[0] = cpool.tile(
                    [EPC, HIDDEN], MMDT, tag="wo_sb", name="wo_sb"
                )
                nc.sync.dma_start(wo[0][:], wo_d[:, :])
                sel2[0] = cpool.tile(
                    [128, EPC], MMDT, tag="sel2_sb", name="sel2_sb"
                )
                nc.sync.dma_start(sel2[0][:], sel2_d[:, :])
            # persistent rowsum staging tile; zeroed once so the unused
            # partitions contribute 0 (not garbage) to the selector matmul
            # (full 128 partitions so the matmul stays in 128x128 mode)
            r2 = cpool.tile([128, TQ], MMDT, tag="r2_sb")
            nc.vector.memset(r2[:], 0.0)
            dummy = cpool.tile([128, TQ], MMDT, tag="dummy_sb")
            nc.vector.memset(dummy[:], 0.0)
            nc.scalar.activation(
                dummy[:, 256:384], dummy[:, 0:128],
                mybir.ActivationFunctionType.Exp,
            )
            dps = ps2.tile([128, TQ], F32, tag="pj", name="dps")
            for _w in range(16):
                nc.tensor.matmul(dps[:], dummy[:, 0:128], dummy[:])

            # Score matmuls run as K=64 row-tiled pairs (2 heads concurrent
            # in the PE array halves); everything else is 128x128 mode.
            def pe(inst):
                return inst

            # ---------------- per-batch projection items ----------------
            qkv = {}  # b -> (q_sb, k_sb, v_sb)

            def make_proj_items(b, t4):
                """Emission items (cost_us, fn) projecting chunk t4 of batch b."""
                if t4 == 0:
                    # q is stored zero-padded per head so score matmuls can
                    # contract over the full 128 partitions (uniform 128x128
                    # PE mode; row-tiled K=64 pairs measured SLOWER because
                    # tiled LDWEIGHTS cannot overlap in-flight matmuls on
                    # the same row group).  The pad rows / ones column are
                    # only written for the first two batches: the pools
                    # rotate over 2 buffers and nothing else ever writes
                    # those regions.
                    qp0 = qkpool.tile([EPC, S], MMDT, tag="q0")
                    qp1 = qkpool.tile([EPC, S], MMDT, tag="q1")
                    k_sb = qkpool.tile([EPC, S], MMDT, tag="k")
                    v_sb = vpool.tile([128, NG, NPAR, 2, VW], MMDT, tag="v")
                    if b < 2:
                        nc.vector.memset(qp0[D:EPC, :], 0.0)
                        nc.vector.memset(qp1[0:D, :], 0.0)
                        nc.vector.memset(v_sb[:, :, :, :, D : D + 1], 1.0)
                    qkv[b] = ((qp0, qp1), k_sb, v_sb)
                qpair, k_sb, v_sb = qkv[b]
                tlo = b * S + t4 * TQ
                ts_ = slice(t4 * TQ, (t4 + 1) * TQ)
                xt_t = xpool.tile([128, DC, TQ], MMDT, tag="xt")
                x8_t = x8pool.tile([128, DC, TQ], FP8, tag="x8")
                for s_ in range(4):
                    nc.sync.dma_start(
                        x8_t[:, 2 * s_ : 2 * s_ + 2, :],
                        xt8_d[s_ * 256 : (s_ + 1) * 256, tlo : tlo + TQ].rearrange(
                            "(dc p) t -> p dc t", p=128
                        ),
                    )
                for s_ in range(4):
                    nc.sync.dma_start(
                        xt_t[:, 2 * s_ : 2 * s_ + 2, :],
                        xt_d[s_ * 256 : (s_ + 1) * 256, tlo : tlo + TQ].rearrange(
                            "(dc p) t -> p dc t", p=128
                        ),
                    )
                st = {}

                def qk_mm(wa, name):
                    # fp8 DoubleRow: contracts 256 per matmul (2 packed
                    # k-subtiles), so 4 matmuls cover the 1024 contraction.
                    def f():
                        pa = ps2.tile([128, TQ], F32, tag="pj", name="pa")
                        for dc in range(0, DC, 2):
                            pe(
                                nc.tensor.matmul(
                                    pa[:], wa[:, dc : dc + 2, :],
                                    x8_t[:, dc : dc + 2, :],
                                    start=(dc == 0), stop=(dc == DC - 2),
                                    perf_mode=mybir.MatmulPerfMode.DoubleRow,
                                )
                            )
                        praw = wpool.tile([EPC, TQ], MMDT, tag="praw", name="praw")
                        nc.vector.tensor_copy(praw[:], pa[:])
                        st[name] = (pa, praw)
                    return f

                def qk_rope(name, bi, dst):
                    def f():
                        pa, praw = st[name]
                        t1 = wpool.tile([EPC, TQ], F32, tag="t1", name="t1")
                        nc.vector.scalar_tensor_tensor(
                            t1[:], pa[:], bq4[:, bi : bi + 1], c1[:, ts_],
                            ALU.add, ALU.mult,
                        )
                        pb = ps2.tile([128, TQ], F32, tag="pj", name="pb")
                        pe(nc.tensor.matmul(pb[:], psw[:], praw[:]))
                        t2 = wpool.tile([EPC, TQ], F32, tag="t2", name="t2")
                        nc.vector.scalar_tensor_tensor(
                            t2[:], pb[:], bq4[:, bi + 1 : bi + 2], c2[:, ts_],
                            ALU.add, ALU.mult,
                        )
                        if isinstance(dst, tuple):
                            nc.vector.tensor_add(
                                dst[0][0:D, ts_], t1[0:D, :], t2[0:D, :]
                            )
                            nc.vector.tensor_add(
                                dst[1][D:EPC, ts_], t1[D:EPC, :], t2[D:EPC, :]
                            )
                        else:
                            nc.vector.tensor_add(dst[:, ts_], t1[:], t2[:])
                    return f

                def v_item(gl, par):
                    def f():
                        g = t4 * NGC + gl
                        vp = ps2.tile([128, TQ], F32, tag="pj", name="vp")
                        cs = slice(gl * GK + par * 128, gl * GK + (par + 1) * 128)
                        for dc in range(DC):
                            pe(
                                nc.tensor.matmul(
                                    vp[:, 0:EPC], xt_t[:, dc, cs], wv[:, dc],
                                    start=(dc == 0), stop=(dc == DC - 1),
                                )
                            )
                        nc.vector.tensor_add(
                            v_sb[:, g, par, 0, 0:D], vp[:, 0:D], bvb[:, 0:D]
                        )
                        nc.vector.tensor_add(
                            v_sb[:, g, par, 1, 0:D], vp[:, D:EPC], bvb[:, D:EPC]
                        )
                    return f

                # q/k matmuls precede both ropes so the psw swap matmul of
                # each rope never chases its praw cast (DVE) closely.
                return [
                    (1.2, qk_mm(wq, "q")),
                    (1.2, qk_mm(wk, "k")),
                    (0.7, qk_rope("q", 0, qpair)),
                    (0.7, qk_rope("k", 2, k_sb)),
                    (0.55, v_item(0, 0)),
                    (0.55, v_item(0, 1)),
                    (0.55, v_item(0, 2)),
                    (0.55, v_item(0, 3)),
                ]

            # ---------------- out-projection items ----------------
            def make_out_items(o_sb, b, t4):
                tail = b == B - 1 and t4 == NTQ - 1

                def out1(eb):
                    def f():
                        yp = ps2.tile([128, TQ], F32, tag="pj", name="yp")
                        pe(
                            nc.tensor.matmul(
                                yp[:], wo[0][:, eb * 128 : (eb + 1) * 128], o_sb[:]
                            )
                        )
                        y_sb = ypool.tile([128, TQ], BF16, tag="ysb", name="ysb")
                        if tail and eb % 2:
                            nc.scalar.copy(y_sb[:], yp[:])
                        else:
                            nc.vector.tensor_copy(y_sb[:], yp[:])
                        nc.sync.dma_start(
                            yt_d[
                                eb * 128 : (eb + 1) * 128,
                                b * S + t4 * TQ : b * S + (t4 + 1) * TQ,
                            ],
                            y_sb[:],
                        )
                    return f

                return [(0.3, out1(eb)) for eb in range(8)]

            # ---------------- attention ----------------
            pv_queue = []  # (v_sb, g, e_t, o0, o1, first, last, (b, t4))
            out_pending = []  # (o_sb, b, t4) normalized chunks awaiting out-proj
            inject = []  # [delay_windows, fn] deferred emission (norm finish)

            def norm_finish(ck, o0, o1):
                b, t4 = ck

                def f():
                    bp = ps2.tile([128, TQ], F32, tag="pj", name="bp")
                    pe(nc.tensor.matmul(bp[:], sel2[0][:], r2[:]))
                    rec = wpool.tile([128, TQ], F32, tag="rec", name="rec")
                    nc.vector.reciprocal_approx_fast(rec[:], bp[:])
                    o_sb = wpool.tile([128, TQ], MMDT, tag="osb", name="osb")
                    nc.vector.tensor_tensor(
                        o_sb[0:D, :], o0[0:D, :], rec[0:D, :], ALU.mult
                    )
                    nc.vector.tensor_tensor(
                        o_sb[D:EPC, :], o1[0:D, :], rec[D:EPC, :], ALU.mult
                    )
                    out_pending.append((o_sb, b, t4))

                return f

            def emit_pv(ent):
                v_sb, g, e_t, o0, o1, first, last, ck = ent
                for h, ps in ((0, o0), (1, o1)):
                    for par in range(NPAR):
                        pe(
                            nc.tensor.matmul(
                                ps[0 : D + 1, :],
                                v_sb[:, g, par, h, 0 : D + 1],
                                e_t[:, par, h, :],
                                start=(first and par == 0),
                                stop=(last and par == NPAR - 1),
                            )
                        )
                if last:
                    # rowsum staging now; the selector matmul + reciprocal +
                    # normalize run a window later so the PE never waits on
                    # the DVE copies.
                    nc.vector.tensor_copy(r2[0:1, :], o0[D : D + 1, :])
                    nc.vector.tensor_copy(r2[32:33, :], o1[D : D + 1, :])
                    inject.append([1, norm_finish(ck, o0, o1)])

            def run_inject():
                for ent in inject:
                    ent[0] -= 1
                fire = [ent for ent in inject if ent[0] <= 0]
                inject[:] = [ent for ent in inject if ent[0] > 0]
                for ent in fire:
                    ent[1]()

            def attention_chunk(b, t4, items, group_budget=0.9):
                qpair, k_sb, v_sb = qkv[b]
                qs = slice(t4 * TQ, (t4 + 1) * TQ)
                o0 = psO.tile([128, TQ], F32, tag="o0", name="o0")
                o1 = psO.tile([128, TQ], F32, tag="o1", name="o1")
                for g in range(NG):
                    kb = g * GK
                    e_t = epool.tile([128, NPAR, 2, TQ], MMDT, tag="e", name="e")
                    for par in range(NPAR):
                        ks = slice(kb + par * 128, kb + (par + 1) * 128)
                        sp = psA.tile([128, 2, TQ], F32, tag="sp", name="sp")
                        for h in (0, 1):
                            pe(
                                nc.tensor.matmul(
                                    sp[:, h, :], k_sb[:, ks], qpair[h][:, qs]
                                )
                            )
                        nc.scalar.activation(
                            e_t[:, par, :, :], sp[:, :, :],
                            mybir.ActivationFunctionType.Exp,
                        )
                    pv_queue.append(
                        (v_sb, g, e_t, o0, o1, g == 0, g == NG - 1, (b, t4))
                    )
                    while len(pv_queue) > 2:
                        emit_pv(pv_queue.pop(0))
                    run_inject()
                    budget = group_budget
                    while items and budget > 0:
                        cst, f = items.pop(0)
                        budget -= cst
                        f()

            # ---------------- program ----------------
            # prologue: batch-0 k/v for all chunks + q of chunk 0 (the fill
            # phase is DMA-bound, so attention cannot usefully start any
            # earlier).  The q projections of chunks 1-3 are deferred into
            # the first attention chunks' item streams.
            deferred = []
            for t4 in range(NTQ):
                pitems = make_proj_items(0, t4)
                if t4 == 0:
                    for _, f in pitems:
                        f()
                else:
                    deferred += pitems[:1] + pitems[2:3]
                    for _, f in pitems[1:2] + pitems[3:]:
                        f()
            load_tail_consts()

            def interleave(a, b):
                out = []
                n = max(len(a), len(b))
                for i in range(n):
                    if i < len(a):
                        out.append(a[i])
                    if i < len(b):
                        out.append(b[i])
                return out

            for b in range(B):
                for t4 in range(NTQ):
                    items = []
                    if deferred:
                        items += [deferred.pop(0), deferred.pop(0)]
                    oitems = (
                        make_out_items(*out_pending.pop(0))
                        if out_pending
                        else []
                    )
                    pitems = (
                        make_proj_items(b + 1, t4) if b + 1 < B else []
                    )
                    items += interleave(oitems, pitems)
                    attention_chunk(b, t4, items)
                    for _, f in items:
                        f()

            while pv_queue:
                emit_pv(pv_queue.pop(0))
            for ent in inject:
                ent[1]()
            inject.clear()
            while out_pending:
                for _, f in make_out_items(*out_pending.pop(0)):
                    f()

    nc.compile()
    return nc


def host_prep(x, Wq, bq, Wk, bk, Wv, bv, Wo, bo):
    """Build the 8 per-core input maps from the full-size inputs."""
    B, S, _ = x.shape
    T = B * S
    perm = token_perm(T)
    xflat = x.reshape(T, HIDDEN).T[:, perm]
    xt = to_mm(xflat)
    xt8 = to_fp8(xflat)

    # rope tables in INTERLEAVED head layout: row 2j and 2j+1 share
    # frequency j.  q'[2j] = q[2j] c_j - q[2j+1] s_j ;
    # q'[2j+1] = q[2j] s_j + q[2j+1] c_j.  With swap() exchanging rows
    # 2j <-> 2j+1:  q' = q * C1 + swap(q) * C2,
    # C1[2j]=C1[2j+1]=c_j, C2[2j]=-s_j, C2[2j+1]=+s_j.
    j = np.arange(D // 2)
    inv_freq = 1.0 / (ROPE_BASE ** (2 * j / D))
    t = np.arange(S, dtype=np.float64)
    fr = np.outer(t, inv_freq)  # [S, 32]
    cf = np.cos(fr).T  # [32, S]
    sf = np.sin(fr).T
    c1h = np.repeat(cf, 2, axis=0)  # [64, S]
    c2h = np.empty((D, S))
    c2h[0::2] = -sf
    c2h[1::2] = sf
    perm_s = token_perm(S)
    c1 = to_mm(np.tile(c1h, (HPC, 1))[:, perm_s] / S8)
    c2 = to_mm(np.tile(c2h, (HPC, 1))[:, perm_s] / S8)

    # adjacent-pair swap permutation (within the 128 local rows)
    swp = np.arange(EPC)
    swp = swp ^ 1  # 2j <-> 2j+1
    psw = np.zeros((EPC, EPC), np.float32)
    psw[swp, np.arange(EPC)] = 1.0

    sel2 = np.zeros((128, EPC), np.float32)
    sel2[0, 0:D] = 1.0
    sel2[32, D : 2 * D] = 1.0

    scale = 1.0 / np.sqrt(D)
    in_maps = []
    for c in range(NCORES):
        rows = slice(c * EPC, (c + 1) * EPC)
        Rq, Rk, Rv = Wq[rows], Wk[rows], Wv[rows]
        bqc, bkc, bvc = bq[rows], bk[rows], bv[rows]
        m = {
            "xt": xt,
            "xt8": xt8,
            "wq": to_fp8((Rq * (scale * S8)).T),
            "wk": to_fp8(Rk.T * S8),
            "wv": to_mm(Rv.T),
            "wo": to_mm(Wo[:, rows].T),
            "psw": to_mm(psw),
            "bq4": np.ascontiguousarray(
                (
                    np.stack(
                        [bqc * scale, bqc[swp] * scale, bkc, bkc[swp]], 1
                    )
                    * S8
                ).astype(np.float32)
            ),
            "bvb": np.ascontiguousarray(
                np.tile(bvc[None, :], (EPC, 1)).astype(np.float32)
            ),
            "c1": c1,
            "c2": c2,
            "sel2": to_mm(sel2),
        }
        in_maps.append(m)
    return in_maps


_NC_CACHE = {}


def _get_nc(B, S):
    key = (B, S)
    if key not in _NC_CACHE:
        _NC_CACHE[key] = build_nc(B, S)
    return _NC_CACHE[key]


def run_cores(in_maps, B, S, trace=False):
    nc = _get_nc(B, S)
    return run_bass_kernel_spmd(
        nc, in_maps, core_ids=list(range(NCORES)), trace=trace
    )


def gather(results, bo, B, S):
    acc = results[0]["yt"].astype(np.float32)
    for c in range(1, NCORES):
        acc = acc + results[c]["yt"]
    T = B * S
    perm = token_perm(T)
    y = np.empty_like(acc.T)
    y[perm] = acc.T
    y = y + bo[None, :]
    return np.ascontiguousarray(y.reshape(B, S, HIDDEN).astype(np.float32))


def kernel(x, Wq, bq, Wk, bk, Wv, bv, Wo, bo):
    x = np.asarray(x, np.float32)
    B, S, _ = x.shape
    in_maps = host_prep(
        x,
        np.asarray(Wq, np.float32), np.asarray(bq, np.float32),
        np.asarray(Wk, np.float32), np.asarray(bk, np.float32),
        np.asarray(Wv, np.float32), np.asarray(bv, np.float32),
        np.asarray(Wo, np.float32), np.asarray(bo, np.float32),
    )
    res = run_cores(in_maps, B, S, trace=False)
    return gather(res.results, np.asarray(bo, np.float32), B, S)



# revision 33
# speedup vs baseline: 1.0046x; 1.0046x over previous
"""Multi-head attention with RoPE on 8 Trainium2 NeuronCores.

Strategy: tensor-parallel over heads (16 heads / 8 cores = 2 heads per
core). Each core computes its 2 heads' q/k/v projections, RoPE, full
(non-causal) softmax attention, and a partial output projection over its
128-dim slice of the attention output; the host sums the 8 partial
outputs and adds the output bias.

Schedule/precision notes:
 - Keys are processed in groups of 256, parity-split (tokens are
   host-permuted so each 256-token group is stored evens-then-odds).
   The 4 score matmuls of a group land in one 4-bank PSUM tile and are
   consumed by a single 2048-element exp on the ACT engine (fewer,
   larger activations).
 - PV lags the exp stream by 2 groups; projections for the next batch
   and the out-projection of the previous chunk are sliced into the
   window of each group so the PE never idles on ACT.
 - scores are computed transposed (S^T [tk, tq]) so exp(S^T) feeds the
   PV matmul directly.  Softmax max-subtraction is skipped (scores are
   O(1)); the denominator comes from a ones column appended to V, is
   broadcast via a small selector matmul, and the reciprocal multiply
   commutes with the out-projection.
 - q/k projections run in fp8e4m3 with perf_mode=DoubleRow (256-deep
   contraction per matmul, halving those matmuls); the fp8 quantization
   noise only perturbs attention scores, where near-uniform softmax over
   2048 keys washes it out.  v / scores / PV / out-proj stay bf16 with
   fp32 PSUM accumulation.
 - q is stored zero-padded per head so score matmuls contract over the
   full 128 partitions in uniform 128x128 PE mode (row-tiled K=64 pairs
   measured slower: tiled LDWEIGHTS cannot overlap in-flight matmuls on
   the same row group).
 - output is written bf16 (halves write DMA); the host gather upcasts.
"""

import sys

for _p in ("/opt/trn_rl_repo",):
    if _p not in sys.path:
        sys.path.append(_p)

import numpy as np

import concourse.bacc as bacc
import concourse.bass as bass
import concourse.mybir as mybir
import concourse.tile as tile
from concourse.bass import _add_dep_helper
from concourse.bass_utils import run_bass_kernel_spmd

F32 = mybir.dt.float32
BF16 = mybir.dt.bfloat16
FP8 = mybir.dt.float8e4
MMDT = BF16
ALU = mybir.AluOpType

HIDDEN = 1024
HEADS = 16
D = 64
ROPE_BASE = 10000.0
NCORES = 8
HPC = HEADS // NCORES  # heads per core = 2
EPC = HPC * D  # out dims per core = 128
B_FULL, S_FULL = 4, 2048

TQ = 512  # query-chunk width
GK = 256  # key-group width (2 parities x 128)
VW = 80  # v storage width per (group, parity, head) slot


import ml_dtypes


def to_mm(a):
    """Convert an fp32 array to the matmul operand dtype (bf16, RNE)."""
    return np.ascontiguousarray(np.asarray(a, np.float32).astype(ml_dtypes.bfloat16))


def to_fp8(a):
    """Convert an fp32 array to fp8e4m3 for DoubleRow matmuls."""
    return np.ascontiguousarray(
        np.asarray(a, np.float32).astype(ml_dtypes.float8_e4m3)
    )


# fp8 weight pre-scale: q/k projection weights are tiny (~1/32 / ~1/256
# after the attention scale), far below fp8e4m3's normal range.  Weights
# and biases are scaled up by S8 on the host; the rope cos/sin tables are
# divided by S8 so the rope output lands back at the true scale.
S8 = 256.0


def token_perm(T):
    """Per-256 group even/odd interleave permutation. perm[p] = source token."""
    return np.arange(T).reshape(-1, 128, 2).transpose(0, 2, 1).reshape(-1)


def build_nc(B, S):
    """Build the per-core Bass program (SPMD: all cores run this)."""
    nc = bacc.Bacc(None, target_bir_lowering=False)
    T = B * S
    NTQ = S // TQ  # query chunks per batch = 4
    NG = S // GK  # key groups per batch = 8
    NGC = TQ // GK  # key groups per query chunk = 2
    DC = HIDDEN // 128  # contraction chunks = 8

    xt_d = nc.dram_tensor("xt", [HIDDEN, T], MMDT, kind="ExternalInput")
    xt8_d = nc.dram_tensor("xt8", [HIDDEN, T], FP8, kind="ExternalInput")
    wq_d = nc.dram_tensor("wq", [HIDDEN, EPC], FP8, kind="ExternalInput")
    wk_d = nc.dram_tensor("wk", [HIDDEN, EPC], FP8, kind="ExternalInput")
    psw_d = nc.dram_tensor("psw", [EPC, EPC], MMDT, kind="ExternalInput")
    wv_d = nc.dram_tensor("wv", [HIDDEN, EPC], MMDT, kind="ExternalInput")
    wo_d = nc.dram_tensor("wo", [EPC, HIDDEN], MMDT, kind="ExternalInput")
    bq4_d = nc.dram_tensor("bq4", [EPC, 4], F32, kind="ExternalInput")
    bvb_d = nc.dram_tensor("bvb", [EPC, EPC], F32, kind="ExternalInput")
    c1_d = nc.dram_tensor("c1", [EPC, S], BF16, kind="ExternalInput")
    c2_d = nc.dram_tensor("c2", [EPC, S], BF16, kind="ExternalInput")
    sel2_d = nc.dram_tensor("sel2", [128, EPC], MMDT, kind="ExternalInput")
    yt_d = nc.dram_tensor("yt", [HIDDEN, T], BF16, kind="ExternalOutput")

    with tile.TileContext(nc) as tc:
        with (
            tc.tile_pool(name="const", bufs=1) as cpool,
            tc.tile_pool(name="xin", bufs=3) as xpool,
            tc.tile_pool(name="x8in", bufs=3) as x8pool,
            tc.tile_pool(name="qk", bufs=2) as qkpool,
            tc.tile_pool(name="vsb", bufs=2) as vpool,
            tc.tile_pool(name="esb", bufs=6) as epool,
            tc.tile_pool(name="work", bufs=2) as wpool,
            tc.tile_pool(name="yout", bufs=6) as ypool,
            tc.tile_pool(name="psA", bufs=2, space="PSUM") as psA,
            tc.tile_pool(name="psO", bufs=1, space="PSUM") as psO,
            tc.tile_pool(name="ps2", bufs=2, space="PSUM") as ps2,
        ):
            # ---- constants (resident all kernel) ----
            # wo/sel2 are loaded after the prologue emission (they are not
            # needed until the first out-projection ~25us in) so their DMA
            # does not compete with the startup-critical loads.
            def _ldw(dram, dt=MMDT):
                t = cpool.tile([128, DC, EPC], dt, tag=dram.name + "_sb")
                nc.sync.dma_start(t[:], dram[:, :].rearrange("(dc p) m -> p dc m", p=128))
                return t

            wq = _ldw(wq_d, FP8)
            bq4 = cpool.tile([EPC, 4], F32, tag="bq4_sb")
            nc.sync.dma_start(bq4[:], bq4_d[:, :])
            c1 = cpool.tile([EPC, S], BF16, tag="c1_sb")
            c2 = cpool.tile([EPC, S], BF16, tag="c2_sb")
            for s_ in range(4):
                cs_ = slice(s_ * S // 4, (s_ + 1) * S // 4)
                nc.sync.dma_start(c1[:, cs_], c1_d[:, cs_])
                nc.sync.dma_start(c2[:, cs_], c2_d[:, cs_])
            wk = _ldw(wk_d, FP8)
            wv = _ldw(wv_d)
            psw = cpool.tile([EPC, EPC], MMDT, tag="psw_sb")
            nc.sync.dma_start(psw[:], psw_d[:, :])
            bvb = cpool.tile([EPC, EPC], F32, tag="bvb_sb")
            nc.sync.dma_start(bvb[:], bvb_d[:, :])
            wo = [None]
            sel2 = [None]

            def load_tail_consts():
                wo[0] = cpool.tile(
                    [EPC, HIDDEN], MMDT, tag="wo_sb", name="wo_sb"
                )
                nc.sync.dma_start(wo[0][:], wo_d[:, :])
                sel2[0] = cpool.tile(
                    [128, EPC], MMDT, tag="sel2_sb", name="sel2_sb"
                )
                nc.sync.dma_start(sel2[0][:], sel2_d[:, :])
            # persistent rowsum staging tile; zeroed once so the unused
            # partitions contribute 0 (not garbage) to the selector matmul
            # (full 128 partitions so the matmul stays in 128x128 mode)
            r2 = cpool.tile([128, TQ], MMDT, tag="r2_sb")
            nc.vector.memset(r2[:], 0.0)
            dummy = cpool.tile([128, TQ], MMDT, tag="dummy_sb")
            nc.vector.memset(dummy[:], 0.0)
            nc.scalar.activation(
                dummy[:, 256:384], dummy[:, 0:128],
                mybir.ActivationFunctionType.Exp,
            )
            dps = ps2.tile([128, TQ], F32, tag="pj", name="dps")
            for _w in range(16):
                nc.tensor.matmul(dps[:], dummy[:, 0:128], dummy[:])

            # Score matmuls run as K=64 row-tiled pairs (2 heads concurrent
            # in the PE array halves); everything else is 128x128 mode.
            def pe(inst):
                return inst

            # ---------------- per-batch projection items ----------------
            qkv = {}  # b -> (q_sb, k_sb, v_sb)

            def make_proj_items(b, t4):
                """Emission items (cost_us, fn) projecting chunk t4 of batch b."""
                if t4 == 0:
                    # q is stored zero-padded per head so score matmuls can
                    # contract over the full 128 partitions (uniform 128x128
                    # PE mode; row-tiled K=64 pairs measured SLOWER because
                    # tiled LDWEIGHTS cannot overlap in-flight matmuls on
                    # the same row group).  The pad rows / ones column are
                    # only written for the first two batches: the pools
                    # rotate over 2 buffers and nothing else ever writes
                    # those regions.
                    qp0 = qkpool.tile([EPC, S], MMDT, tag="q0")
                    qp1 = qkpool.tile([EPC, S], MMDT, tag="q1")
                    k_sb = qkpool.tile([EPC, S], MMDT, tag="k")
                    v_sb = vpool.tile([128, NG, 2, 2, VW], MMDT, tag="v")
                    if b < 2:
                        nc.vector.memset(qp0[D:EPC, :], 0.0)
                        nc.vector.memset(qp1[0:D, :], 0.0)
                        nc.vector.memset(v_sb[:, :, :, :, D : D + 1], 1.0)
                    qkv[b] = ((qp0, qp1), k_sb, v_sb)
                qpair, k_sb, v_sb = qkv[b]
                tlo = b * S + t4 * TQ
                ts_ = slice(t4 * TQ, (t4 + 1) * TQ)
                xt_t = xpool.tile([128, DC, TQ], MMDT, tag="xt")
                x8_t = x8pool.tile([128, DC, TQ], FP8, tag="x8")
                for s_ in range(4):
                    nc.sync.dma_start(
                        x8_t[:, 2 * s_ : 2 * s_ + 2, :],
                        xt8_d[s_ * 256 : (s_ + 1) * 256, tlo : tlo + TQ].rearrange(
                            "(dc p) t -> p dc t", p=128
                        ),
                    )
                for s_ in range(4):
                    nc.sync.dma_start(
                        xt_t[:, 2 * s_ : 2 * s_ + 2, :],
                        xt_d[s_ * 256 : (s_ + 1) * 256, tlo : tlo + TQ].rearrange(
                            "(dc p) t -> p dc t", p=128
                        ),
                    )
                st = {}

                def qk_mm(wa, name):
                    # fp8 DoubleRow: contracts 256 per matmul (2 packed
                    # k-subtiles), so 4 matmuls cover the 1024 contraction.
                    def f():
                        pa = ps2.tile([128, TQ], F32, tag="pj", name="pa")
                        for dc in range(0, DC, 2):
                            pe(
                                nc.tensor.matmul(
                                    pa[:], wa[:, dc : dc + 2, :],
                                    x8_t[:, dc : dc + 2, :],
                                    start=(dc == 0), stop=(dc == DC - 2),
                                    perf_mode=mybir.MatmulPerfMode.DoubleRow,
                                )
                            )
                        praw = wpool.tile([EPC, TQ], MMDT, tag="praw", name="praw")
                        nc.vector.tensor_copy(praw[:], pa[:])
                        st[name] = (pa, praw)
                    return f

                def qk_rope(name, bi, dst):
                    def f():
                        pa, praw = st[name]
                        t1 = wpool.tile([EPC, TQ], F32, tag="t1", name="t1")
                        nc.vector.scalar_tensor_tensor(
                            t1[:], pa[:], bq4[:, bi : bi + 1], c1[:, ts_],
                            ALU.add, ALU.mult,
                        )
                        pb = ps2.tile([128, TQ], F32, tag="pj", name="pb")
                        pe(nc.tensor.matmul(pb[:], psw[:], praw[:]))
                        t2 = wpool.tile([EPC, TQ], F32, tag="t2", name="t2")
                        nc.vector.scalar_tensor_tensor(
                            t2[:], pb[:], bq4[:, bi + 1 : bi + 2], c2[:, ts_],
                            ALU.add, ALU.mult,
                        )
                        if isinstance(dst, tuple):
                            nc.vector.tensor_add(
                                dst[0][0:D, ts_], t1[0:D, :], t2[0:D, :]
                            )
                            nc.vector.tensor_add(
                                dst[1][D:EPC, ts_], t1[D:EPC, :], t2[D:EPC, :]
                            )
                        else:
                            nc.vector.tensor_add(dst[:, ts_], t1[:], t2[:])
                    return f

                def v_item(gl, par):
                    def f():
                        g = t4 * NGC + gl
                        vp = ps2.tile([128, TQ], F32, tag="pj", name="vp")
                        cs = slice(gl * GK + par * 128, gl * GK + (par + 1) * 128)
                        for dc in range(DC):
                            pe(
                                nc.tensor.matmul(
                                    vp[:, 0:EPC], xt_t[:, dc, cs], wv[:, dc],
                                    start=(dc == 0), stop=(dc == DC - 1),
                                )
                            )
                        nc.vector.tensor_add(
                            v_sb[:, g, par, 0, 0:D], vp[:, 0:D], bvb[:, 0:D]
                        )
                        nc.vector.tensor_add(
                            v_sb[:, g, par, 1, 0:D], vp[:, D:EPC], bvb[:, D:EPC]
                        )
                    return f

                # q/k matmuls precede both ropes so the psw swap matmul of
                # each rope never chases its praw cast (DVE) closely.
                return [
                    (1.2, qk_mm(wq, "q")),
                    (1.2, qk_mm(wk, "k")),
                    (0.7, qk_rope("q", 0, qpair)),
                    (0.7, qk_rope("k", 2, k_sb)),
                    (0.55, v_item(0, 0)),
                    (0.55, v_item(0, 1)),
                    (0.55, v_item(1, 0)),
                    (0.55, v_item(1, 1)),
                ]

            # ---------------- out-projection items ----------------
            def make_out_items(o_sb, b, t4):
                tail = b == B - 1 and t4 == NTQ - 1

                def out1(eb):
                    def f():
                        yp = ps2.tile([128, TQ], F32, tag="pj", name="yp")
                        pe(
                            nc.tensor.matmul(
                                yp[:], wo[0][:, eb * 128 : (eb + 1) * 128], o_sb[:]
                            )
                        )
                        y_sb = ypool.tile([128, TQ], BF16, tag="ysb", name="ysb")
                        if tail and eb % 2:
                            nc.scalar.copy(y_sb[:], yp[:])
                        else:
                            nc.vector.tensor_copy(y_sb[:], yp[:])
                        nc.sync.dma_start(
                            yt_d[
                                eb * 128 : (eb + 1) * 128,
                                b * S + t4 * TQ : b * S + (t4 + 1) * TQ,
                            ],
                            y_sb[:],
                        )
                    return f

                return [(0.3, out1(eb)) for eb in range(8)]

            # ---------------- attention ----------------
            pv_queue = []  # (v_sb, g, e_t, o0, o1, first, last, (b, t4))
            out_pending = []  # (o_sb, b, t4) normalized chunks awaiting out-proj
            inject = []  # [delay_windows, fn] deferred emission (norm finish)

            def norm_finish(ck, o0, o1):
                b, t4 = ck

                def f():
                    bp = ps2.tile([128, TQ], F32, tag="pj", name="bp")
                    pe(nc.tensor.matmul(bp[:], sel2[0][:], r2[:]))
                    rec = wpool.tile([128, TQ], F32, tag="rec", name="rec")
                    nc.vector.reciprocal_approx_fast(rec[:], bp[:])
                    o_sb = wpool.tile([128, TQ], MMDT, tag="osb", name="osb")
                    nc.vector.tensor_tensor(
                        o_sb[0:D, :], o0[0:D, :], rec[0:D, :], ALU.mult
                    )
                    nc.vector.tensor_tensor(
                        o_sb[D:EPC, :], o1[0:D, :], rec[D:EPC, :], ALU.mult
                    )
                    out_pending.append((o_sb, b, t4))

                return f

            def emit_pv(ent):
                v_sb, g, e_t, o0, o1, first, last, ck = ent
                for h, ps in ((0, o0), (1, o1)):
                    for par in (0, 1):
                        pe(
                            nc.tensor.matmul(
                                ps[0 : D + 1, :],
                                v_sb[:, g, par, h, 0 : D + 1],
                                e_t[:, par, h, :],
                                start=(first and par == 0),
                                stop=(last and par == 1),
                            )
                        )
                if last:
                    # rowsum staging now; the selector matmul + reciprocal +
                    # normalize run a window later so the PE never waits on
                    # the DVE copies.
                    nc.vector.tensor_copy(r2[0:1, :], o0[D : D + 1, :])
                    nc.vector.tensor_copy(r2[32:33, :], o1[D : D + 1, :])
                    inject.append([1, norm_finish(ck, o0, o1)])

            def run_inject():
                for ent in inject:
                    ent[0] -= 1
                fire = [ent for ent in inject if ent[0] <= 0]
                inject[:] = [ent for ent in inject if ent[0] > 0]
                for ent in fire:
                    ent[1]()

            def attention_chunk(b, t4, items, group_budget=0.9):
                qpair, k_sb, v_sb = qkv[b]
                qs = slice(t4 * TQ, (t4 + 1) * TQ)
                o0 = psO.tile([128, TQ], F32, tag="o0", name="o0")
                o1 = psO.tile([128, TQ], F32, tag="o1", name="o1")
                for g in range(NG):
                    kb = g * GK
                    e_t = epool.tile([128, 2, 2, TQ], MMDT, tag="e", name="e")
                    for par in (0, 1):
                        ks = slice(kb + par * 128, kb + (par + 1) * 128)
                        sp = psA.tile([128, 2, TQ], F32, tag="sp", name="sp")
                        for h in (0, 1):
                            pe(
                                nc.tensor.matmul(
                                    sp[:, h, :], k_sb[:, ks], qpair[h][:, qs]
                                )
                            )
                        nc.scalar.activation(
                            e_t[:, par, :, :], sp[:, :, :],
                            mybir.ActivationFunctionType.Exp,
                        )
                    pv_queue.append(
                        (v_sb, g, e_t, o0, o1, g == 0, g == NG - 1, (b, t4))
                    )
                    while len(pv_queue) > 2:
                        emit_pv(pv_queue.pop(0))
                    run_inject()
                    budget = group_budget
                    while items and budget > 0:
                        cst, f = items.pop(0)
                        budget -= cst
                        f()

            # ---------------- program ----------------
            # prologue: batch-0 k/v for all chunks + q of chunk 0 (the fill
            # phase is DMA-bound, so attention cannot usefully start any
            # earlier).  The q projections of chunks 1-3 are deferred into
            # the first attention chunks' item streams.
            deferred = []
            for t4 in range(NTQ):
                pitems = make_proj_items(0, t4)
                if t4 == 0:
                    for _, f in pitems:
                        f()
                else:
                    deferred += pitems[:1] + pitems[2:3]
                    for _, f in pitems[1:2] + pitems[3:]:
                        f()
            load_tail_consts()

            def interleave(a, b):
                out = []
                n = max(len(a), len(b))
                for i in range(n):
                    if i < len(a):
                        out.append(a[i])
                    if i < len(b):
                        out.append(b[i])
                return out

            for b in range(B):
                for t4 in range(NTQ):
                    items = []
                    if deferred:
                        items += [deferred.pop(0), deferred.pop(0)]
                    oitems = (
                        make_out_items(*out_pending.pop(0))
                        if out_pending
                        else []
                    )
                    pitems = (
                        make_proj_items(b + 1, t4) if b + 1 < B else []
                    )
                    items += interleave(oitems, pitems)
                    attention_chunk(b, t4, items)
                    for _, f in items:
                        f()

            while pv_queue:
                emit_pv(pv_queue.pop(0))
            for ent in inject:
                ent[1]()
            inject.clear()
            while out_pending:
                for _, f in make_out_items(*out_pending.pop(0)):
                    f()

    nc.compile()
    return nc


def host_prep(x, Wq, bq, Wk, bk, Wv, bv, Wo, bo):
    """Build the 8 per-core input maps from the full-size inputs."""
    B, S, _ = x.shape
    T = B * S
    perm = token_perm(T)
    xflat = x.reshape(T, HIDDEN).T[:, perm]
    xt = to_mm(xflat)
    xt8 = to_fp8(xflat)

    # rope tables in INTERLEAVED head layout: row 2j and 2j+1 share
    # frequency j.  q'[2j] = q[2j] c_j - q[2j+1] s_j ;
    # q'[2j+1] = q[2j] s_j + q[2j+1] c_j.  With swap() exchanging rows
    # 2j <-> 2j+1:  q' = q * C1 + swap(q) * C2,
    # C1[2j]=C1[2j+1]=c_j, C2[2j]=-s_j, C2[2j+1]=+s_j.
    j = np.arange(D // 2)
    inv_freq = 1.0 / (ROPE_BASE ** (2 * j / D))
    t = np.arange(S, dtype=np.float64)
    fr = np.outer(t, inv_freq)  # [S, 32]
    cf = np.cos(fr).T  # [32, S]
    sf = np.sin(fr).T
    c1h = np.repeat(cf, 2, axis=0)  # [64, S]
    c2h = np.empty((D, S))
    c2h[0::2] = -sf
    c2h[1::2] = sf
    perm_s = token_perm(S)
    c1 = to_mm(np.tile(c1h, (HPC, 1))[:, perm_s] / S8)
    c2 = to_mm(np.tile(c2h, (HPC, 1))[:, perm_s] / S8)

    # adjacent-pair swap permutation (within the 128 local rows)
    swp = np.arange(EPC)
    swp = swp ^ 1  # 2j <-> 2j+1
    psw = np.zeros((EPC, EPC), np.float32)
    psw[swp, np.arange(EPC)] = 1.0

    sel2 = np.zeros((128, EPC), np.float32)
    sel2[0, 0:D] = 1.0
    sel2[32, D : 2 * D] = 1.0

    scale = 1.0 / np.sqrt(D)
    in_maps = []
    for c in range(NCORES):
        rows = slice(c * EPC, (c + 1) * EPC)
        Rq, Rk, Rv = Wq[rows], Wk[rows], Wv[rows]
        bqc, bkc, bvc = bq[rows], bk[rows], bv[rows]
        m = {
            "xt": xt,
            "xt8": xt8,
            "wq": to_fp8((Rq * (scale * S8)).T),
            "wk": to_fp8(Rk.T * S8),
            "wv": to_mm(Rv.T),
            "wo": to_mm(Wo[:, rows].T),
            "psw": to_mm(psw),
            "bq4": np.ascontiguousarray(
                (
                    np.stack(
                        [bqc * scale, bqc[swp] * scale, bkc, bkc[swp]], 1
                    )
                    * S8
                ).astype(np.float32)
            ),
            "bvb": np.ascontiguousarray(
                np.tile(bvc[None, :], (EPC, 1)).astype(np.float32)
            ),
            "c1": c1,
            "c2": c2,
            "sel2": to_mm(sel2),
        }
        in_maps.append(m)
    return in_maps


_NC_CACHE = {}


def _get_nc(B, S):
    key = (B, S)
    if key not in _NC_CACHE:
        _NC_CACHE[key] = build_nc(B, S)
    return _NC_CACHE[key]


def run_cores(in_maps, B, S, trace=False):
    nc = _get_nc(B, S)
    return run_bass_kernel_spmd(
        nc, in_maps, core_ids=list(range(NCORES)), trace=trace
    )


def gather(results, bo, B, S):
    acc = results[0]["yt"].astype(np.float32)
    for c in range(1, NCORES):
        acc = acc + results[c]["yt"]
    T = B * S
    perm = token_perm(T)
    y = np.empty_like(acc.T)
    y[perm] = acc.T
    y = y + bo[None, :]
    return np.ascontiguousarray(y.reshape(B, S, HIDDEN).astype(np.float32))


def kernel(x, Wq, bq, Wk, bk, Wv, bv, Wo, bo):
    x = np.asarray(x, np.float32)
    B, S, _ = x.shape
    in_maps = host_prep(
        x,
        np.asarray(Wq, np.float32), np.asarray(bq, np.float32),
        np.asarray(Wk, np.float32), np.asarray(bk, np.float32),
        np.asarray(Wv, np.float32), np.asarray(bv, np.float32),
        np.asarray(Wo, np.float32), np.asarray(bo, np.float32),
    )
    res = run_cores(in_maps, B, S, trace=False)
    return gather(res.results, np.asarray(bo, np.float32), B, S)

